# revision 14
# baseline (speedup 1.0000x reference)
"""Fused attention kernel for Trainium2 (Bass/Tile), SPMD over 8 NeuronCores.

Problem: B=4, D=64, S=4096 fp32 attention
    A = softmax_k(K^T Q / sqrt(D));  R = V A;  out = concat(R, Q) on channel dim.

Sharding: 8 cores = 4 batches x 2 query-halves (Sq=2048 per core).

v4 design (v1 baseline 74.5us):
  * k-pair-outer loop over PAIRS of q-tiles: every stationary operand
    (K-tile, V-tile, ones) serves 2 matmuls. tile_legalize emits an
    InstLdweights per matmul unconditionally, so a post-schedule dedup
    pass deletes weight loads that are identical to the still-live load
    on a non-clobbered PE-array region — the second matmul of each pair
    then streams back-to-back on the already-loaded weights.
  * S = K^T Q row-tiled 2x (k-even rows 0:64, k-odd 64:128) sharing the
    duplicated-Q stream; per-(pair,qtile) s-psum tile [128, 1024] =
    (k-even 512 | k-odd 512), ring of 2.
  * exp: ONE [128,1024] instruction per (pair,qtile) — the engines have
    ~0.5-0.6us fixed cost per instruction, so coarse is much cheaper.
    Engine alternates with (g+t) parity for balance. Scores span
    e^{+-16} on this dataset which exceeds fp16 range entirely, so E is
    bf16: ACT LUT exp -> bf16, DVE one-instruction Schraudolph integer
    exp (u16 = trunc(s*16*log2e + const) = bf16 bits, ~3% sawtooth).
  * R = V E col-tiled 2x; Z = ones^T E col-tiled 4x; weights reused
    across the q-tile pair via the dedup pass.
  * No on-device softmax divide: raw R-halves + Z rows out, host divides.
  * PSUM: 2x s-tiles (2 banks each) + 2 r + 2 z = 8 banks exactly.
"""

import sys

sys.path.insert(0, "/opt/trn_rl_repo")

import numpy as np  # noqa: E402
import ml_dtypes  # noqa: E402

B, D, S = 4, 64, 4096
NCORES = 8
SQ = S * B // NCORES  # 2048 queries per core
QT = 512              # q-tile width
KT = 128              # k-tile width
NQT = SQ // QT        # 4 q-tiles per core
NKT = S // KT         # 32 k-tiles
NPAIR = NKT // 2      # 16 k-tile pairs
NH = NQT // 2         # 2 q-tile pairs (halves)
LOG2E = 1.4426950408889634
# DVE Schraudolph to bf16 bits: u16 = trunc(s * A_SCH + B_SCH)
A_SCH = 128.0 * LOG2E / 8.0
B_SCH = 128.0 * 127.0 - 4.0
LAG = 2               # k-pairs between S/exp and the consuming R matmul
N_WARM = 32           # dummy matmuls to pre-warm the PE HAM clock gate
DEDUP = True          # run the ldweights dedup pass

_nc_cache = None


def _dedupe_ldweights(nc, mybir):
    """Delete InstLdweights whose (weights, position) match the live load
    on a non-clobbered array region; move their deps to the consuming
    matmul and remap references. Runs after tile scheduling, before
    nc.compile()."""
    n_del = 0
    for func in nc.m.functions:
        for block in func.blocks:
            insts = list(block.instructions)
            new = []
            live = []   # (r0, r1, c0, c1, key, name) newest last
            rename = {}
            for idx, inst in enumerate(insts):
                if isinstance(inst, mybir.InstLdweights):
                    ap = inst.ins[0]
                    tp = tuple(inst.tile_position or (0, 0))
                    ts = tuple(inst.tile_size or (128, 128))
                    key = (
                        ap.memref, ap.offset, str(ap.ap), str(ap.dtype),
                        tp, ts, str(inst.perf_mode), bool(inst.is_transpose),
                    )
                    r0, c0 = tp
                    r1, c1 = r0 + ts[0], c0 + ts[1]
                    dup = False
                    for e in reversed(live):
                        if not (r1 <= e[0] or e[1] <= r0 or c1 <= e[2] or e[3] <= c0):
                            dup = e[4] == key
                            break
                    if dup:
                        nxt = None
                        for j in range(idx + 1, len(insts)):
                            if isinstance(insts[j], mybir.InstMatmult):
                                nxt = insts[j]
                                break
                        assert nxt is not None
                        nxt.merge_dependencies_from(inst)
                        rename[inst.name] = nxt.name
                        n_del += 1
                        continue
                    # this load clobbers overlapping regions
                    live = [
                        e for e in live
                        if (r1 <= e[0] or e[1] <= r0 or c1 <= e[2] or e[3] <= c0)
                    ]
                    live.append((r0, r1, c0, c1, key, inst.name))
                new.append(inst)
            if rename:
                for inst in new:
                    inst.remap_dependency_names(rename)
                block.instructions = new
                assert len(block.instructions) == len(new)
    return n_del


def _build():
    global _nc_cache
    if _nc_cache is not None:
        return _nc_cache
    import concourse.tile as tile
    from concourse import bacc, mybir

    nc = bacc.Bacc(None, target_bir_lowering=False)
    f32 = mybir.dt.float32
    f16 = mybir.dt.float16
    u16 = mybir.dt.uint16
    bf16 = mybir.dt.bfloat16

    kst = nc.dram_tensor("kst", [2 * D, NPAIR * KT], f16, kind="ExternalInput")
    qrep = nc.dram_tensor("qrep", [2 * D, SQ], f16, kind="ExternalInput")
    vtin = nc.dram_tensor("vtin", [KT, NKT * D], bf16, kind="ExternalInput")
    # raw R-partial halves (even k-tiles on rows 0:64, odd on 64:128)
    out_r = nc.dram_tensor("out_r", [D, SQ], f32, kind="ExternalOutput")
    # Z partial rows (from PSUM partitions 0/32/64/96); host sums + divides
    out_z = nc.dram_tensor("out_z", [2, SQ], f32, kind="ExternalOutput")

    with tile.TileContext(nc) as tc:
        with (
            tc.tile_pool(name="singles", bufs=1) as singles,
            tc.tile_pool(name="e", bufs=12) as pe_pool,
            tc.tile_pool(name="ro", bufs=2) as pro,
            tc.tile_pool(name="zo", bufs=2) as pzo,
            tc.tile_pool(name="ps_s", bufs=3, space="PSUM") as ps_s,
            tc.tile_pool(name="ps_r", bufs=1, space="PSUM") as ps_r,
            tc.tile_pool(name="ps_z", bufs=1, space="PSUM") as ps_z,
        ):
            k_sb = singles.tile([2 * D, NPAIR * KT], f16)
            q_sb = singles.tile([2 * D, SQ], f16)
            vt_sb = singles.tile([KT, NKT * D], bf16)
            bias_sb = singles.tile([KT, 1], f32)
            warm_sb = singles.tile([KT, KT], f16)
            onesb = singles.tile([KT, 1], bf16)
            nc.vector.memset(bias_sb, 0.0)
            nc.vector.memset(warm_sb, 0.0)
            nc.vector.memset(onesb, 1.0)

            from concourse.tile_rust import add_dep_helper

            # Wave 1 (~0.6MB): first work units. Wave 2: everything else,
            # held back so its SDMA traffic doesn't delay the pipeline head.
            nc.sync.dma_start(out=q_sb[:, : 2 * QT], in_=qrep[:, : 2 * QT])
            nc.sync.dma_start(out=k_sb[:, : 4 * KT], in_=kst[:, : 4 * KT])
            d_kh = nc.gpsimd.dma_start(out=vt_sb[:, : 8 * D], in_=vtin[:, : 8 * D])
            w2 = [
                nc.sync.dma_start(out=k_sb[:, 4 * KT :], in_=kst[:, 4 * KT :]),
                nc.gpsimd.dma_start(out=vt_sb[:, 8 * D :], in_=vtin[:, 8 * D :]),
                nc.sync.dma_start(out=q_sb[:, 2 * QT :], in_=qrep[:, 2 * QT :]),
            ]
            for bulk in w2:
                add_dep_helper(
                    bulk.ins, d_kh.ins, sync=True,
                    reason="bulk input DMA after first work unit",
                )

            vt = vt_sb.rearrange("p (j d) -> p j d", j=NKT)

            # HAM warm-up: PE activity with no DMA dependency so the clock
            # gate approaches 8/8 before the real matmuls arrive. Writes into
            # the first s-psum ring slot (WAW-serialized with the real S).
            first_s = None
            if N_WARM:
                first_s = ps_s.tile([KT, 2 * QT], f32, name="s_ps")
                for _ in range(N_WARM):
                    nc.tensor.matmul(
                        first_s[:, :KT], warm_sb, warm_sb, start=True, stop=True
                    )

            Exp = mybir.ActivationFunctionType.Exp
            mult = mybir.AluOpType.mult
            add = mybir.AluOpType.add

            for h in range(NH):
                t0c = 2 * h * QT          # column base of q-tile t0
                r_ps = ps_r.tile([2 * D, QT], f32, name="r_ps")
                z_ps = ps_z.tile([KT, QT], f32, name="z_ps")
                e_of = {}

                def emit_R(gg, r_ps=r_ps, z_ps=z_ps, e_of=e_of):
                    j0 = 2 * gg
                    st = gg == 0
                    sp = gg == NPAIR - 1
                    # One R bank: t0 -> partitions 0:64 via col position 0,
                    # t1 -> 64:128 via position 64. V_even contributions
                    # start the per-element groups, V_odd accumulates on
                    # top (start=False) and carries the stop.
                    nc.tensor.matmul(
                        r_ps[0:D, :], vt[:, j0, 0:D], e_of[gg][0][:, :QT],
                        start=st, stop=False, tile_position=(0, 0),
                        skip_group_check=True,
                    )
                    nc.tensor.matmul(
                        r_ps[D : 2 * D, :], vt[:, j0, 0:D], e_of[gg][1][:, :QT],
                        start=st, stop=False, tile_position=(0, D),
                        skip_group_check=True,
                    )
                    nc.tensor.matmul(
                        r_ps[0:D, :], vt[:, j0 + 1, 0:D], e_of[gg][0][:, QT:],
                        start=False, stop=sp, tile_position=(0, 0),
                        skip_group_check=True,
                    )
                    nc.tensor.matmul(
                        r_ps[D : 2 * D, :], vt[:, j0 + 1, 0:D], e_of[gg][1][:, QT:],
                        start=False, stop=sp, tile_position=(0, D),
                        skip_group_check=True,
                    )
                    if gg % 2 == 1:
                        # Z quad over k-tiles 4b..4b+3 in one bank:
                        # positions 0/32 accumulate q-tile t0, 64/96 t1;
                        # each position chain runs 2 k-tiles sequentially
                        # off one ones-load.
                        b = gg // 2
                        zst = b == 0
                        zsp = b == NPAIR // 2 - 1
                        for i in range(4):
                            t = i // 2
                            p = 2 * b + (i % 2)
                            for half in range(2):
                                cs = slice(0, QT) if half == 0 else slice(QT, 2 * QT)
                                nc.tensor.matmul(
                                    z_ps[32 * i : 32 * i + 1, :], onesb,
                                    e_of[p][t][:, cs],
                                    start=zst and half == 0,
                                    stop=zsp and half == 1,
                                    tile_position=(0, 32 * i),
                                    skip_group_check=True,
                                )
                        del e_of[2 * b], e_of[2 * b + 1]

                for g in range(NPAIR):
                    # S pair: two (A|B) psum tiles, one per q-tile
                    if first_s is not None:
                        st0, first_s = first_s, None
                    else:
                        st0 = ps_s.tile([KT, 2 * QT], f32, name="s_ps")
                    st1 = ps_s.tile([KT, 2 * QT], f32, name="s_ps")
                    gc = slice(g * KT, (g + 1) * KT)
                    qs0 = slice(t0c, t0c + QT)
                    qs1 = slice(t0c + QT, t0c + 2 * QT)
                    nc.tensor.matmul(
                        st0[:, :QT], k_sb[0:D, gc], q_sb[0:D, qs0],
                        start=True, stop=True, tile_position=(0, 0),
                    )
                    nc.tensor.matmul(
                        st0[:, QT:], k_sb[D : 2 * D, gc], q_sb[D : 2 * D, qs0],
                        start=True, stop=True, tile_position=(D, 0),
                    )
                    nc.tensor.matmul(
                        st1[:, :QT], k_sb[0:D, gc], q_sb[0:D, qs1],
                        start=True, stop=True, tile_position=(0, 0),
                    )
                    nc.tensor.matmul(
                        st1[:, QT:], k_sb[D : 2 * D, gc], q_sb[D : 2 * D, qs1],
                        start=True, stop=True, tile_position=(D, 0),
                    )

                    # exp: one [128,1024] instruction per q-tile; fixed
                    # engines (t0 -> ACT, t1 -> DVE) so each ring slot's
                    # free-latency lines up with the PE's natural order
                    e0 = pe_pool.tile([KT, 2 * QT], bf16, tag="e")
                    e1 = pe_pool.tile([KT, 2 * QT], bf16, tag="e")
                    for t, (e_t, s_t) in enumerate(((e0, st0), (e1, st1))):
                        if t == 1:
                            nc.scalar.activation(
                                out=e_t, in_=s_t, func=Exp, scale=0.125,
                                bias=bias_sb,
                            )
                        else:
                            nc.vector.tensor_scalar(
                                e_t.bitcast(u16), s_t, A_SCH, B_SCH, mult, add
                            )
                    e_of[g] = (e0, e1)

                    if g >= LAG:
                        emit_R(g - LAG)
                for gg in range(NPAIR - LAG, NPAIR):
                    emit_R(gg)

                # evacuate R and Z (one copy each; ACT gets R, DVE Z)
                cols0 = slice(t0c, t0c + QT)
                cols1 = slice(t0c + QT, t0c + 2 * QT)
                r_sb = pro.tile([2 * D, QT], f32, tag="r_sb")
                z_sb = pzo.tile([3 * 32 + 1, QT], f32, tag="z_sb")
                nc.scalar.copy(out=r_sb, in_=r_ps)
                nc.vector.tensor_scalar(
                    z_sb, z_ps[0 : 3 * 32 + 1, :], 1.0, 0.0, mult, add
                )
                nc.sync.dma_start(out=out_r[:, cols0], in_=r_sb[0:D, :])
                nc.sync.dma_start(out=out_r[:, cols1], in_=r_sb[D : 2 * D, :])
                for i in range(4):
                    t, row = i // 2, i % 2
                    nc.sync.dma_start(
                        out=out_z[row : row + 1, cols1 if t else cols0],
                        in_=z_sb[32 * i : 32 * i + 1, :],
                    )

    if DEDUP:
        from concourse import mybir as _mb
        n = _dedupe_ldweights(nc, _mb)
        print(f"[kernel] ldweights dedup removed {n}")
    nc.compile()
    _nc_cache = nc
    return nc


def _in_maps(K, V, Q):
    K = np.asarray(K, dtype=np.float32)
    V = np.asarray(V, dtype=np.float32)
    Q = np.asarray(Q, dtype=np.float32)
    maps = []
    for c in range(NCORES):
        b, h = c // 2, c % 2
        # kst: pair-interleaved K tiles: rows 0:64 = even k-tiles, 64:128 = odd
        k16 = K[b].astype(np.float16).reshape(D, NKT, KT)
        kst = np.concatenate([k16[:, 0::2], k16[:, 1::2]], axis=0)  # [128,16,128]
        qhi = Q[b, :, h * SQ : (h + 1) * SQ].astype(np.float16)
        qrep = np.concatenate([qhi, qhi], axis=0)  # [128, SQ]
        # V'^T tiles: vt[p, j, d] = V[b, d, KT*j + p], bf16
        vtb = np.ascontiguousarray(
            V[b].T.reshape(NKT, KT, D).transpose(1, 0, 2)
        ).astype(ml_dtypes.bfloat16)
        maps.append(
            {
                "kst": np.ascontiguousarray(kst.reshape(2 * D, NPAIR * KT)),
                "qrep": np.ascontiguousarray(qrep),
                "vtin": np.ascontiguousarray(vtb.reshape(KT, NKT * D)),
            }
        )
    return maps


def _run(K, V, Q, trace=False):
    from concourse.bass_utils import run_bass_kernel_spmd

    nc = _build()
    res = run_bass_kernel_spmd(
        nc, _in_maps(K, V, Q), list(range(NCORES)), trace=trace
    )
    Q = np.asarray(Q, dtype=np.float32)
    out = np.empty((B, 2 * D, S), dtype=np.float32)
    out[:, D : 2 * D, :] = Q
    for c in range(NCORES):
        b, h = c // 2, c % 2
        rr = res.results[c]["out_r"].astype(np.float64)
        zz = res.results[c]["out_z"].astype(np.float64)
        den = zz.sum(axis=0)
        out[b, 0:D, h * SQ : (h + 1) * SQ] = (rr / den[None, :]).astype(np.float32)
    return out, res


def kernel(K, V, Q):
    out, _ = _run(K, V, Q, trace=False)
    return out


# revision 15
# speedup vs baseline: 1.0738x; 1.0738x over previous
"""Fused attention kernel for Trainium2 (Bass/Tile), SPMD over 8 NeuronCores.

Problem: B=4, D=64, S=4096 fp32 attention
    A = softmax_k(K^T Q / sqrt(D));  R = V A;  out = concat(R, Q) on channel dim.

Sharding: 8 cores = 4 batches x 2 query-halves (Sq=2048 per core).

v4 design (v1 baseline 74.5us):
  * k-pair-outer loop over PAIRS of q-tiles: every stationary operand
    (K-tile, V-tile, ones) serves 2 matmuls. tile_legalize emits an
    InstLdweights per matmul unconditionally, so a post-schedule dedup
    pass deletes weight loads that are identical to the still-live load
    on a non-clobbered PE-array region — the second matmul of each pair
    then streams back-to-back on the already-loaded weights.
  * S = K^T Q row-tiled 2x (k-even rows 0:64, k-odd 64:128) sharing the
    duplicated-Q stream; per-(pair,qtile) s-psum tile [128, 1024] =
    (k-even 512 | k-odd 512), ring of 2.
  * exp: ONE [128,1024] instruction per (pair,qtile) — the engines have
    ~0.5-0.6us fixed cost per instruction, so coarse is much cheaper.
    Engine alternates with (g+t) parity for balance. Scores span
    e^{+-16} on this dataset which exceeds fp16 range entirely, so E is
    bf16: ACT LUT exp -> bf16, DVE one-instruction Schraudolph integer
    exp (u16 = trunc(s*16*log2e + const) = bf16 bits, ~3% sawtooth).
  * R = V E col-tiled 2x; Z = ones^T E col-tiled 4x; weights reused
    across the q-tile pair via the dedup pass.
  * No on-device softmax divide: raw R-halves + Z rows out, host divides.
  * PSUM: 2x s-tiles (2 banks each) + 2 r + 2 z = 8 banks exactly.
"""

import sys

sys.path.insert(0, "/opt/trn_rl_repo")

import numpy as np  # noqa: E402
import ml_dtypes  # noqa: E402

B, D, S = 4, 64, 4096
NCORES = 8
SQ = S * B // NCORES  # 2048 queries per core
QT = 512              # q-tile width
KT = 128              # k-tile width
NQT = SQ // QT        # 4 q-tiles per core
NKT = S // KT         # 32 k-tiles
NPAIR = NKT // 2      # 16 k-tile pairs
NH = NQT // 2         # 2 q-tile pairs (halves)
LOG2E = 1.4426950408889634
# DVE Schraudolph to bf16 bits: u16 = trunc(s * A_SCH + B_SCH)
A_SCH = 128.0 * LOG2E / 8.0
B_SCH = 128.0 * 127.0 - 4.0
LAG = 2               # k-pairs between S/exp and the consuming R matmul
N_WARM = 32           # dummy matmuls to pre-warm the PE HAM clock gate
DEDUP = True          # run the ldweights dedup pass

_nc_cache = None


def _dedupe_ldweights(nc, mybir):
    """Delete InstLdweights whose (weights, position) match the live load
    on a non-clobbered array region; move their deps to the consuming
    matmul and remap references. Runs after tile scheduling, before
    nc.compile()."""
    n_del = 0
    for func in nc.m.functions:
        for block in func.blocks:
            insts = list(block.instructions)
            new = []
            live = []   # (r0, r1, c0, c1, key, name) newest last
            rename = {}
            for idx, inst in enumerate(insts):
                if isinstance(inst, mybir.InstLdweights):
                    ap = inst.ins[0]
                    tp = tuple(inst.tile_position or (0, 0))
                    ts = tuple(inst.tile_size or (128, 128))
                    key = (
                        ap.memref, ap.offset, str(ap.ap), str(ap.dtype),
                        tp, ts, str(inst.perf_mode), bool(inst.is_transpose),
                    )
                    r0, c0 = tp
                    r1, c1 = r0 + ts[0], c0 + ts[1]
                    dup = False
                    for e in reversed(live):
                        if not (r1 <= e[0] or e[1] <= r0 or c1 <= e[2] or e[3] <= c0):
                            dup = e[4] == key
                            break
                    if dup:
                        nxt = None
                        for j in range(idx + 1, len(insts)):
                            if isinstance(insts[j], mybir.InstMatmult):
                                nxt = insts[j]
                                break
                        assert nxt is not None
                        nxt.merge_dependencies_from(inst)
                        rename[inst.name] = nxt.name
                        n_del += 1
                        continue
                    # this load clobbers overlapping regions
                    live = [
                        e for e in live
                        if (r1 <= e[0] or e[1] <= r0 or c1 <= e[2] or e[3] <= c0)
                    ]
                    live.append((r0, r1, c0, c1, key, inst.name))
                new.append(inst)
            if rename:
                for inst in new:
                    inst.remap_dependency_names(rename)
                block.instructions = new
                assert len(block.instructions) == len(new)
    return n_del


def _build():
    global _nc_cache
    if _nc_cache is not None:
        return _nc_cache
    import concourse.tile as tile
    from concourse import bacc, mybir

    nc = bacc.Bacc(None, target_bir_lowering=False)
    f32 = mybir.dt.float32
    f16 = mybir.dt.float16
    u16 = mybir.dt.uint16
    bf16 = mybir.dt.bfloat16

    kst = nc.dram_tensor("kst", [2 * D, NPAIR * KT], f16, kind="ExternalInput")
    qrep = nc.dram_tensor("qrep", [2 * D, SQ], f16, kind="ExternalInput")
    vtin = nc.dram_tensor("vtin", [KT, NKT * D], bf16, kind="ExternalInput")
    # raw R-partial halves (even k-tiles on rows 0:64, odd on 64:128)
    out_r = nc.dram_tensor("out_r", [D, SQ], f32, kind="ExternalOutput")
    # Z partial rows (from PSUM partitions 0/32/64/96); host sums + divides
    out_z = nc.dram_tensor("out_z", [2, SQ], f32, kind="ExternalOutput")

    with tile.TileContext(nc) as tc:
        with (
            tc.tile_pool(name="singles", bufs=1) as singles,
            tc.tile_pool(name="e", bufs=12) as pe_pool,
            tc.tile_pool(name="ro", bufs=2) as pro,
            tc.tile_pool(name="zo", bufs=2) as pzo,
            tc.tile_pool(name="ps_s", bufs=3, space="PSUM") as ps_s,
            tc.tile_pool(name="ps_r", bufs=1, space="PSUM") as ps_r,
            tc.tile_pool(name="ps_z", bufs=1, space="PSUM") as ps_z,
        ):
            k_sb = singles.tile([2 * D, NPAIR * KT], f16)
            q_sb = singles.tile([2 * D, SQ], f16)
            vt_sb = singles.tile([KT, NKT * D], bf16)
            bias_sb = singles.tile([KT, 1], f32)
            warm_sb = singles.tile([KT, KT], f16)
            onesb = singles.tile([KT, 1], bf16)
            nc.vector.memset(bias_sb, 0.0)
            nc.vector.memset(warm_sb, 0.0)
            nc.vector.memset(onesb, 1.0)

            from concourse.tile_rust import add_dep_helper

            # Wave 1 (~0.6MB): first work units. Wave 2: everything else,
            # held back so its SDMA traffic doesn't delay the pipeline head.
            nc.sync.dma_start(out=q_sb[:, : 2 * QT], in_=qrep[:, : 2 * QT])
            nc.sync.dma_start(out=k_sb[:, : 4 * KT], in_=kst[:, : 4 * KT])
            d_kh = nc.gpsimd.dma_start(out=vt_sb[:, : 8 * D], in_=vtin[:, : 8 * D])
            w2 = [
                nc.sync.dma_start(out=k_sb[:, 4 * KT :], in_=kst[:, 4 * KT :]),
                nc.gpsimd.dma_start(out=vt_sb[:, 8 * D :], in_=vtin[:, 8 * D :]),
                nc.sync.dma_start(out=q_sb[:, 2 * QT :], in_=qrep[:, 2 * QT :]),
            ]
            for bulk in w2:
                add_dep_helper(
                    bulk.ins, d_kh.ins, sync=True,
                    reason="bulk input DMA after first work unit",
                )

            vt = vt_sb.rearrange("p (j d) -> p j d", j=NKT)

            # HAM warm-up: PE activity with no DMA dependency so the clock
            # gate approaches 8/8 before the real matmuls arrive. Writes into
            # the first s-psum ring slot (WAW-serialized with the real S).
            first_s = None
            if N_WARM:
                first_s = ps_s.tile([KT, 2 * QT], f32, name="s_ps")
                for _ in range(N_WARM):
                    nc.tensor.matmul(
                        first_s[:, :KT], warm_sb, warm_sb, start=True, stop=True
                    )

            Exp = mybir.ActivationFunctionType.Exp
            mult = mybir.AluOpType.mult
            add = mybir.AluOpType.add

            for h in range(NH):
                t0c = 2 * h * QT          # column base of q-tile t0
                r_ps = ps_r.tile([2 * D, QT], f32, name="r_ps")
                z_ps = ps_z.tile([KT, QT], f32, name="z_ps")
                e_of = {}

                def emit_R(gg, r_ps=r_ps, z_ps=z_ps, e_of=e_of):
                    j0 = 2 * gg
                    st = gg == 0
                    sp = gg == NPAIR - 1
                    # One R bank: t0 -> partitions 0:64 via col position 0,
                    # t1 -> 64:128 via position 64. V_even contributions
                    # start the per-element groups, V_odd accumulates on
                    # top (start=False) and carries the stop.
                    nc.tensor.matmul(
                        r_ps[0:D, :], vt[:, j0, 0:D], e_of[gg][0][:, :QT],
                        start=st, stop=False, tile_position=(0, 0),
                        skip_group_check=True,
                    )
                    nc.tensor.matmul(
                        r_ps[D : 2 * D, :], vt[:, j0, 0:D], e_of[gg][1][:, :QT],
                        start=st, stop=False, tile_position=(0, D),
                        skip_group_check=True,
                    )
                    nc.tensor.matmul(
                        r_ps[0:D, :], vt[:, j0 + 1, 0:D], e_of[gg][0][:, QT:],
                        start=False, stop=sp, tile_position=(0, 0),
                        skip_group_check=True,
                    )
                    nc.tensor.matmul(
                        r_ps[D : 2 * D, :], vt[:, j0 + 1, 0:D], e_of[gg][1][:, QT:],
                        start=False, stop=sp, tile_position=(0, D),
                        skip_group_check=True,
                    )
                    if gg % 2 == 1:
                        # Z quad over k-tiles 4b..4b+3 in one bank:
                        # positions 0/32 accumulate q-tile t0, 64/96 t1;
                        # each position chain runs 2 k-tiles sequentially
                        # off one ones-load.
                        b = gg // 2
                        zst = b == 0
                        zsp = b == NPAIR // 2 - 1
                        for i in range(4):
                            t = i // 2
                            p = 2 * b + (i % 2)
                            for half in range(2):
                                cs = slice(0, QT) if half == 0 else slice(QT, 2 * QT)
                                nc.tensor.matmul(
                                    z_ps[32 * i : 32 * i + 1, :], onesb,
                                    e_of[p][t][:, cs],
                                    start=zst and half == 0,
                                    stop=zsp and half == 1,
                                    tile_position=(0, 32 * i),
                                    skip_group_check=True,
                                )
                        del e_of[2 * b], e_of[2 * b + 1]

                for g in range(NPAIR):
                    # S pair: two (A|B) psum tiles, one per q-tile
                    if first_s is not None:
                        st0, first_s = first_s, None
                    else:
                        st0 = ps_s.tile([KT, 2 * QT], f32, name="s_ps")
                    st1 = ps_s.tile([KT, 2 * QT], f32, name="s_ps")
                    gc = slice(g * KT, (g + 1) * KT)
                    qs0 = slice(t0c, t0c + QT)
                    qs1 = slice(t0c + QT, t0c + 2 * QT)
                    nc.tensor.matmul(
                        st0[:, :QT], k_sb[0:D, gc], q_sb[0:D, qs0],
                        start=True, stop=True, tile_position=(0, 0),
                    )
                    nc.tensor.matmul(
                        st0[:, QT:], k_sb[D : 2 * D, gc], q_sb[D : 2 * D, qs0],
                        start=True, stop=True, tile_position=(D, 0),
                    )
                    nc.tensor.matmul(
                        st1[:, :QT], k_sb[0:D, gc], q_sb[0:D, qs1],
                        start=True, stop=True, tile_position=(0, 0),
                    )
                    nc.tensor.matmul(
                        st1[:, QT:], k_sb[D : 2 * D, gc], q_sb[D : 2 * D, qs1],
                        start=True, stop=True, tile_position=(D, 0),
                    )

                    # exp: one [128,1024] instruction per q-tile; fixed
                    # engines (t0 -> ACT, t1 -> DVE) so each ring slot's
                    # free-latency lines up with the PE's natural order
                    e0 = pe_pool.tile([KT, 2 * QT], bf16, tag="e")
                    e1 = pe_pool.tile([KT, 2 * QT], bf16, tag="e")
                    for t, (e_t, s_t) in enumerate(((e0, st0), (e1, st1))):
                        if t == 0:
                            nc.scalar.activation(
                                out=e_t, in_=s_t, func=Exp, scale=0.125,
                                bias=bias_sb,
                            )
                        else:
                            nc.vector.tensor_scalar(
                                e_t.bitcast(u16), s_t, A_SCH, B_SCH, mult, add
                            )
                    e_of[g] = (e0, e1)

                    if g >= LAG:
                        emit_R(g - LAG)
                for gg in range(NPAIR - LAG, NPAIR):
                    emit_R(gg)

                # evacuate R and Z (one copy each; ACT gets R, DVE Z)
                cols0 = slice(t0c, t0c + QT)
                cols1 = slice(t0c + QT, t0c + 2 * QT)
                r_sb = pro.tile([2 * D, QT], f32, tag="r_sb")
                z_sb = pzo.tile([3 * 32 + 1, QT], f32, tag="z_sb")
                nc.scalar.copy(out=r_sb, in_=r_ps)
                nc.vector.tensor_scalar(
                    z_sb, z_ps[0 : 3 * 32 + 1, :], 1.0, 0.0, mult, add
                )
                nc.sync.dma_start(out=out_r[:, cols0], in_=r_sb[0:D, :])
                nc.sync.dma_start(out=out_r[:, cols1], in_=r_sb[D : 2 * D, :])
                for i in range(4):
                    t, row = i // 2, i % 2
                    nc.sync.dma_start(
                        out=out_z[row : row + 1, cols1 if t else cols0],
                        in_=z_sb[32 * i : 32 * i + 1, :],
                    )

    if DEDUP:
        from concourse import mybir as _mb
        n = _dedupe_ldweights(nc, _mb)
        print(f"[kernel] ldweights dedup removed {n}")
    nc.compile()
    _nc_cache = nc
    return nc


def _in_maps(K, V, Q):
    K = np.asarray(K, dtype=np.float32)
    V = np.asarray(V, dtype=np.float32)
    Q = np.asarray(Q, dtype=np.float32)
    maps = []
    for c in range(NCORES):
        b, h = c // 2, c % 2
        # kst: pair-interleaved K tiles: rows 0:64 = even k-tiles, 64:128 = odd
        k16 = K[b].astype(np.float16).reshape(D, NKT, KT)
        kst = np.concatenate([k16[:, 0::2], k16[:, 1::2]], axis=0)  # [128,16,128]
        qhi = Q[b, :, h * SQ : (h + 1) * SQ].astype(np.float16)
        qrep = np.concatenate([qhi, qhi], axis=0)  # [128, SQ]
        # V'^T tiles: vt[p, j, d] = V[b, d, KT*j + p], bf16
        vtb = np.ascontiguousarray(
            V[b].T.reshape(NKT, KT, D).transpose(1, 0, 2)
        ).astype(ml_dtypes.bfloat16)
        maps.append(
            {
                "kst": np.ascontiguousarray(kst.reshape(2 * D, NPAIR * KT)),
                "qrep": np.ascontiguousarray(qrep),
                "vtin": np.ascontiguousarray(vtb.reshape(KT, NKT * D)),
            }
        )
    return maps


def _run(K, V, Q, trace=False):
    from concourse.bass_utils import run_bass_kernel_spmd

    nc = _build()
    res = run_bass_kernel_spmd(
        nc, _in_maps(K, V, Q), list(range(NCORES)), trace=trace
    )
    Q = np.asarray(Q, dtype=np.float32)
    out = np.empty((B, 2 * D, S), dtype=np.float32)
    out[:, D : 2 * D, :] = Q
    for c in range(NCORES):
        b, h = c // 2, c % 2
        rr = res.results[c]["out_r"].astype(np.float64)
        zz = res.results[c]["out_z"].astype(np.float64)
        den = zz.sum(axis=0)
        out[b, 0:D, h * SQ : (h + 1) * SQ] = (rr / den[None, :]).astype(np.float32)
    return out, res


def kernel(K, V, Q):
    out, _ = _run(K, V, Q, trace=False)
    return out


# revision 18
# speedup vs baseline: 1.2043x; 1.1215x over previous
"""Fused attention kernel for Trainium2 (Bass/Tile), SPMD over 8 NeuronCores.

Problem: B=4, D=64, S=4096 fp32 attention
    A = softmax_k(K^T Q / sqrt(D));  R = V A;  out = concat(R, Q) on channel dim.

Sharding: 8 cores = 4 batches x 2 query-halves (Sq=2048 per core).

v5 design, ~69-74us HW (v1 baseline 74.5us on a rested device):
  * k-pair-outer loop over PAIRS of q-tiles: every stationary operand
    (K-tile, V-tile, ones) serves 2+ matmuls. tile_legalize emits an
    InstLdweights per matmul unconditionally, so a post-schedule dedup
    pass deletes weight loads identical to the still-live load on a
    non-clobbered PE-array region — the extra matmuls then stream
    back-to-back on already-loaded weights (~150 loads removed).
  * S = K^T Q row-tiled 2x (k-even rows 0:64, k-odd 64:128) sharing the
    duplicated-Q stream; per-(pair,qtile) s-psum tile [128, 1024] =
    (k-even 512 | k-odd 512), ring of THREE tiles so the S -> exp -> S
    slot-recycle loop has a spare tile of slack.
  * exp: ONE [128,1024] instruction per (pair,qtile) — ACT/DVE have
    ~0.5-0.6us fixed cost per instruction, so coarse granularity wins.
    Fixed engines: t0 -> ACT (LUT exp), t1 -> DVE (one-instruction
    Schraudolph integer exp: u16 = trunc(s*16*log2e + const) = bf16
    bits, ~3% sawtooth). Scores span e^{+-16} on this dataset — beyond
    fp16's entire range — so E must be bf16 (8 exponent bits).
  * R = V E with V_even starting each per-element accumulation group
    and V_odd accumulating on top: ONE psum bank holds both q-tiles
    (t0 -> partitions 0:64 via col position 0, t1 -> 64:128 via 64),
    so no host-side half-summing and half the output DMA.
  * Z = ones^T E in ONE bank: col positions 0/32 accumulate q-tile t0,
    64/96 t1, each position chain running 2 k-tiles per ones-load.
  * No on-device softmax divide: raw R + Z partial rows out, host adds
    the 2 Z rows and divides.
  * PSUM: 3x s-tiles (2 banks each) + 1 r + 1 z = 8 banks exactly.
"""

import sys

sys.path.insert(0, "/opt/trn_rl_repo")

import numpy as np  # noqa: E402
import ml_dtypes  # noqa: E402

B, D, S = 4, 64, 4096
NCORES = 8
SQ = S * B // NCORES  # 2048 queries per core
QT = 512              # q-tile width
KT = 128              # k-tile width
NQT = SQ // QT        # 4 q-tiles per core
NKT = S // KT         # 32 k-tiles
NPAIR = NKT // 2      # 16 k-tile pairs
NH = NQT // 2         # 2 q-tile pairs (halves)
LOG2E = 1.4426950408889634
# DVE Schraudolph to bf16 bits: u16 = trunc(s * A_SCH + B_SCH)
A_SCH = 128.0 * LOG2E / 8.0
B_SCH = 128.0 * 127.0 - 4.0
LAG = 2               # k-pairs between S/exp and the consuming R matmul
N_WARM = 32           # dummy matmuls to pre-warm the PE HAM clock gate
DEDUP = True          # run the ldweights dedup pass

_nc_cache = None


def _dedupe_ldweights(nc, mybir):
    """Delete InstLdweights whose (weights, position) match the live load
    on a non-clobbered array region; move their deps to the consuming
    matmul and remap references. Runs after tile scheduling, before
    nc.compile()."""
    n_del = 0
    for func in nc.m.functions:
        for block in func.blocks:
            insts = list(block.instructions)
            new = []
            live = []   # (r0, r1, c0, c1, key, name) newest last
            rename = {}
            for idx, inst in enumerate(insts):
                if isinstance(inst, mybir.InstLdweights):
                    ap = inst.ins[0]
                    tp = tuple(inst.tile_position or (0, 0))
                    ts = tuple(inst.tile_size or (128, 128))
                    key = (
                        ap.memref, ap.offset, str(ap.ap), str(ap.dtype),
                        tp, ts, str(inst.perf_mode), bool(inst.is_transpose),
                    )
                    r0, c0 = tp
                    r1, c1 = r0 + ts[0], c0 + ts[1]
                    dup = False
                    for e in reversed(live):
                        if not (r1 <= e[0] or e[1] <= r0 or c1 <= e[2] or e[3] <= c0):
                            dup = e[4] == key
                            break
                    if dup:
                        nxt = None
                        for j in range(idx + 1, len(insts)):
                            if isinstance(insts[j], mybir.InstMatmult):
                                nxt = insts[j]
                                break
                        assert nxt is not None
                        nxt.merge_dependencies_from(inst)
                        rename[inst.name] = nxt.name
                        n_del += 1
                        continue
                    # this load clobbers overlapping regions
                    live = [
                        e for e in live
                        if (r1 <= e[0] or e[1] <= r0 or c1 <= e[2] or e[3] <= c0)
                    ]
                    live.append((r0, r1, c0, c1, key, inst.name))
                new.append(inst)
            if rename:
                for inst in new:
                    inst.remap_dependency_names(rename)
                block.instructions = new
                assert len(block.instructions) == len(new)
    return n_del


def _build():
    global _nc_cache
    if _nc_cache is not None:
        return _nc_cache
    import concourse.tile as tile
    from concourse import bacc, mybir

    nc = bacc.Bacc(None, target_bir_lowering=False)
    # Keep matmul waits OFF the ldweights: the weight load then streams
    # into the array while the psum-slot wait is pending, instead of
    # serializing behind it.
    nc.move_matmul_waits_to_ldweights = lambda: None
    f32 = mybir.dt.float32
    f16 = mybir.dt.float16
    u16 = mybir.dt.uint16
    bf16 = mybir.dt.bfloat16

    kst = nc.dram_tensor("kst", [2 * D, NPAIR * KT], f16, kind="ExternalInput")
    qrep = nc.dram_tensor("qrep", [2 * D, SQ], f16, kind="ExternalInput")
    vtin = nc.dram_tensor("vtin", [KT, NKT * D], bf16, kind="ExternalInput")
    # raw R-partial halves (even k-tiles on rows 0:64, odd on 64:128)
    out_r = nc.dram_tensor("out_r", [D, SQ], f32, kind="ExternalOutput")
    # Z partial rows (from PSUM partitions 0/32/64/96); host sums + divides
    out_z = nc.dram_tensor("out_z", [2, SQ], f32, kind="ExternalOutput")

    with tile.TileContext(nc) as tc:
        with (
            tc.tile_pool(name="singles", bufs=1) as singles,
            tc.tile_pool(name="e", bufs=12) as pe_pool,
            tc.tile_pool(name="ro", bufs=2) as pro,
            tc.tile_pool(name="zo", bufs=2) as pzo,
            tc.tile_pool(name="ps_s", bufs=3, space="PSUM") as ps_s,
            tc.tile_pool(name="ps_r", bufs=1, space="PSUM") as ps_r,
            tc.tile_pool(name="ps_z", bufs=1, space="PSUM") as ps_z,
        ):
            k_sb = singles.tile([2 * D, NPAIR * KT], f16)
            q_sb = singles.tile([2 * D, SQ], f16)
            vt_sb = singles.tile([KT, NKT * D], bf16)
            bias_sb = singles.tile([KT, 1], f32)
            warm_sb = singles.tile([KT, KT], f16)
            onesb = singles.tile([KT, 1], bf16)
            # gpsimd's queue is empty at the head, so these land ~1us
            # earlier than on DVE and unblock the HAM warm-up matmuls
            nc.gpsimd.memset(warm_sb, 0.0)
            nc.gpsimd.memset(bias_sb, 0.0)
            nc.gpsimd.memset(onesb, 1.0)

            from concourse.tile_rust import add_dep_helper

            # Wave 1 (~0.6MB): first work units. Wave 2: everything else,
            # held back so its SDMA traffic doesn't delay the pipeline head.
            nc.sync.dma_start(out=q_sb[:, : 2 * QT], in_=qrep[:, : 2 * QT])
            nc.sync.dma_start(out=k_sb[:, : 4 * KT], in_=kst[:, : 4 * KT])
            d_kh = nc.gpsimd.dma_start(out=vt_sb[:, : 8 * D], in_=vtin[:, : 8 * D])
            w2 = [
                nc.sync.dma_start(out=k_sb[:, 4 * KT :], in_=kst[:, 4 * KT :]),
                nc.gpsimd.dma_start(out=vt_sb[:, 8 * D :], in_=vtin[:, 8 * D :]),
                nc.sync.dma_start(out=q_sb[:, 2 * QT :], in_=qrep[:, 2 * QT :]),
            ]
            for bulk in w2:
                add_dep_helper(
                    bulk.ins, d_kh.ins, sync=True,
                    reason="bulk input DMA after first work unit",
                )

            vt = vt_sb.rearrange("p (j d) -> p j d", j=NKT)

            # HAM warm-up: PE activity with no DMA dependency so the clock
            # gate approaches 8/8 before the real matmuls arrive. Writes into
            # the first s-psum ring slot (WAW-serialized with the real S).
            first_s = None
            if N_WARM:
                first_s = ps_s.tile([KT, 2 * QT], f32, name="s_ps")
                for _ in range(N_WARM):
                    nc.tensor.matmul(
                        first_s[:, :KT], warm_sb, warm_sb, start=True, stop=True
                    )

            Exp = mybir.ActivationFunctionType.Exp
            mult = mybir.AluOpType.mult
            add = mybir.AluOpType.add

            for h in range(NH):
                t0c = 2 * h * QT          # column base of q-tile t0
                r_ps = ps_r.tile([2 * D, QT], f32, name="r_ps")
                z_ps = ps_z.tile([KT, QT], f32, name="z_ps")
                e_of = {}

                def emit_R(gg, r_ps=r_ps, z_ps=z_ps, e_of=e_of):
                    j0 = 2 * gg
                    st = gg == 0
                    sp = gg == NPAIR - 1
                    # One R bank: t0 -> partitions 0:64 via col position 0,
                    # t1 -> 64:128 via position 64. V_even contributions
                    # start the per-element groups, V_odd accumulates on
                    # top (start=False) and carries the stop.
                    nc.tensor.matmul(
                        r_ps[0:D, :], vt[:, j0, 0:D], e_of[gg][0][:, :QT],
                        start=st, stop=False, tile_position=(0, 0),
                        skip_group_check=True,
                    )
                    nc.tensor.matmul(
                        r_ps[D : 2 * D, :], vt[:, j0, 0:D], e_of[gg][1][:, :QT],
                        start=st, stop=False, tile_position=(0, D),
                        skip_group_check=True,
                    )
                    nc.tensor.matmul(
                        r_ps[0:D, :], vt[:, j0 + 1, 0:D], e_of[gg][0][:, QT:],
                        start=False, stop=sp, tile_position=(0, 0),
                        skip_group_check=True,
                    )
                    nc.tensor.matmul(
                        r_ps[D : 2 * D, :], vt[:, j0 + 1, 0:D], e_of[gg][1][:, QT:],
                        start=False, stop=sp, tile_position=(0, D),
                        skip_group_check=True,
                    )
                    if gg % 2 == 1:
                        # Z quad over k-tiles 4b..4b+3 in one bank:
                        # positions 0/32 accumulate q-tile t0, 64/96 t1;
                        # each position chain runs 2 k-tiles sequentially
                        # off one ones-load.
                        b = gg // 2
                        zst = b == 0
                        zsp = b == NPAIR // 2 - 1
                        for i in range(4):
                            t = i // 2
                            p = 2 * b + (i % 2)
                            for half in range(2):
                                cs = slice(0, QT) if half == 0 else slice(QT, 2 * QT)
                                nc.tensor.matmul(
                                    z_ps[32 * i : 32 * i + 1, :], onesb,
                                    e_of[p][t][:, cs],
                                    start=zst and half == 0,
                                    stop=zsp and half == 1,
                                    tile_position=(0, 32 * i),
                                    skip_group_check=True,
                                )
                        del e_of[2 * b], e_of[2 * b + 1]

                for g in range(NPAIR):
                    # S pair: two (A|B) psum tiles, one per q-tile
                    if first_s is not None:
                        st0, first_s = first_s, None
                    else:
                        st0 = ps_s.tile([KT, 2 * QT], f32, name="s_ps")
                    st1 = ps_s.tile([KT, 2 * QT], f32, name="s_ps")
                    gc = slice(g * KT, (g + 1) * KT)
                    qs0 = slice(t0c, t0c + QT)
                    qs1 = slice(t0c + QT, t0c + 2 * QT)
                    nc.tensor.matmul(
                        st0[:, :QT], k_sb[0:D, gc], q_sb[0:D, qs0],
                        start=True, stop=True, tile_position=(0, 0),
                    )
                    nc.tensor.matmul(
                        st0[:, QT:], k_sb[D : 2 * D, gc], q_sb[D : 2 * D, qs0],
                        start=True, stop=True, tile_position=(D, 0),
                    )
                    nc.tensor.matmul(
                        st1[:, :QT], k_sb[0:D, gc], q_sb[0:D, qs1],
                        start=True, stop=True, tile_position=(0, 0),
                    )
                    nc.tensor.matmul(
                        st1[:, QT:], k_sb[D : 2 * D, gc], q_sb[D : 2 * D, qs1],
                        start=True, stop=True, tile_position=(D, 0),
                    )

                    # exp: one [128,1024] instruction per q-tile; fixed
                    # engines (t0 -> ACT, t1 -> DVE) so each ring slot's
                    # free-latency lines up with the PE's natural order
                    e0 = pe_pool.tile([KT, 2 * QT], bf16, tag="e")
                    e1 = pe_pool.tile([KT, 2 * QT], bf16, tag="e")
                    for t, (e_t, s_t) in enumerate(((e0, st0), (e1, st1))):
                        if t == 0:
                            nc.scalar.activation(
                                out=e_t, in_=s_t, func=Exp, scale=0.125,
                                bias=bias_sb,
                            )
                        else:
                            nc.vector.tensor_scalar(
                                e_t.bitcast(u16), s_t, A_SCH, B_SCH, mult, add
                            )
                    e_of[g] = (e0, e1)

                    if g >= LAG:
                        emit_R(g - LAG)
                for gg in range(NPAIR - LAG, NPAIR):
                    emit_R(gg)

                # evacuate R and Z (one copy each; ACT gets R, DVE Z)
                cols0 = slice(t0c, t0c + QT)
                cols1 = slice(t0c + QT, t0c + 2 * QT)
                r_sb = pro.tile([2 * D, QT], f32, tag="r_sb")
                z_sb = pzo.tile([3 * 32 + 1, QT], f32, tag="z_sb")
                nc.scalar.copy(out=r_sb, in_=r_ps)
                nc.vector.tensor_scalar(
                    z_sb, z_ps[0 : 3 * 32 + 1, :], 1.0, 0.0, mult, add
                )
                nc.sync.dma_start(out=out_r[:, cols0], in_=r_sb[0:D, :])
                nc.sync.dma_start(out=out_r[:, cols1], in_=r_sb[D : 2 * D, :])
                for i in range(4):
                    t, row = i // 2, i % 2
                    nc.sync.dma_start(
                        out=out_z[row : row + 1, cols1 if t else cols0],
                        in_=z_sb[32 * i : 32 * i + 1, :],
                    )

    if DEDUP:
        from concourse import mybir as _mb
        n = _dedupe_ldweights(nc, _mb)
        print(f"[kernel] ldweights dedup removed {n}")
    nc.compile()
    _nc_cache = nc
    return nc


def _in_maps(K, V, Q):
    K = np.asarray(K, dtype=np.float32)
    V = np.asarray(V, dtype=np.float32)
    Q = np.asarray(Q, dtype=np.float32)
    maps = []
    for c in range(NCORES):
        b, h = c // 2, c % 2
        # kst: pair-interleaved K tiles: rows 0:64 = even k-tiles, 64:128 = odd
        k16 = K[b].astype(np.float16).reshape(D, NKT, KT)
        kst = np.concatenate([k16[:, 0::2], k16[:, 1::2]], axis=0)  # [128,16,128]
        qhi = Q[b, :, h * SQ : (h + 1) * SQ].astype(np.float16)
        qrep = np.concatenate([qhi, qhi], axis=0)  # [128, SQ]
        # V'^T tiles: vt[p, j, d] = V[b, d, KT*j + p], bf16
        vtb = np.ascontiguousarray(
            V[b].T.reshape(NKT, KT, D).transpose(1, 0, 2)
        ).astype(ml_dtypes.bfloat16)
        maps.append(
            {
                "kst": np.ascontiguousarray(kst.reshape(2 * D, NPAIR * KT)),
                "qrep": np.ascontiguousarray(qrep),
                "vtin": np.ascontiguousarray(vtb.reshape(KT, NKT * D)),
            }
        )
    return maps


def _run(K, V, Q, trace=False):
    from concourse.bass_utils import run_bass_kernel_spmd

    nc = _build()
    res = run_bass_kernel_spmd(
        nc, _in_maps(K, V, Q), list(range(NCORES)), trace=trace
    )
    Q = np.asarray(Q, dtype=np.float32)
    out = np.empty((B, 2 * D, S), dtype=np.float32)
    out[:, D : 2 * D, :] = Q
    for c in range(NCORES):
        b, h = c // 2, c % 2
        rr = res.results[c]["out_r"].astype(np.float64)
        zz = res.results[c]["out_z"].astype(np.float64)
        den = zz.sum(axis=0)
        out[b, 0:D, h * SQ : (h + 1) * SQ] = (rr / den[None, :]).astype(np.float32)
    return out, res


def kernel(K, V, Q):
    out, _ = _run(K, V, Q, trace=False)
    return out


# revision 19
# speedup vs baseline: 1.2138x; 1.0079x over previous
"""Fused attention kernel for Trainium2 (Bass/Tile), SPMD over 8 NeuronCores.

Problem: B=4, D=64, S=4096 fp32 attention
    A = softmax_k(K^T Q / sqrt(D));  R = V A;  out = concat(R, Q) on channel dim.

Sharding: 8 cores = 4 batches x 2 query-halves (Sq=2048 per core).

v5 design, ~69-74us HW (v1 baseline 74.5us on a rested device):
  * k-pair-outer loop over PAIRS of q-tiles: every stationary operand
    (K-tile, V-tile, ones) serves 2+ matmuls. tile_legalize emits an
    InstLdweights per matmul unconditionally, so a post-schedule dedup
    pass deletes weight loads identical to the still-live load on a
    non-clobbered PE-array region — the extra matmuls then stream
    back-to-back on already-loaded weights (~150 loads removed).
  * S = K^T Q row-tiled 2x (k-even rows 0:64, k-odd 64:128) sharing the
    duplicated-Q stream; per-(pair,qtile) s-psum tile [128, 1024] =
    (k-even 512 | k-odd 512), ring of THREE tiles so the S -> exp -> S
    slot-recycle loop has a spare tile of slack.
  * exp: ONE [128,1024] instruction per (pair,qtile) — ACT/DVE have
    ~0.5-0.6us fixed cost per instruction, so coarse granularity wins.
    Fixed engines: t0 -> ACT (LUT exp), t1 -> DVE (one-instruction
    Schraudolph integer exp: u16 = trunc(s*16*log2e + const) = bf16
    bits, ~3% sawtooth). Scores span e^{+-16} on this dataset — beyond
    fp16's entire range — so E must be bf16 (8 exponent bits).
  * R = V E with V_even starting each per-element accumulation group
    and V_odd accumulating on top: ONE psum bank holds both q-tiles
    (t0 -> partitions 0:64 via col position 0, t1 -> 64:128 via 64),
    so no host-side half-summing and half the output DMA.
  * Z = ones^T E in ONE bank: col positions 0/32 accumulate q-tile t0,
    64/96 t1, each position chain running 2 k-tiles per ones-load.
  * No on-device softmax divide: raw R + Z partial rows out, host adds
    the 2 Z rows and divides.
  * PSUM: 3x s-tiles (2 banks each) + 1 r + 1 z = 8 banks exactly.
"""

import sys

sys.path.insert(0, "/opt/trn_rl_repo")

import numpy as np  # noqa: E402
import ml_dtypes  # noqa: E402

B, D, S = 4, 64, 4096
NCORES = 8
SQ = S * B // NCORES  # 2048 queries per core
QT = 512              # q-tile width
KT = 128              # k-tile width
NQT = SQ // QT        # 4 q-tiles per core
NKT = S // KT         # 32 k-tiles
NPAIR = NKT // 2      # 16 k-tile pairs
NH = NQT // 2         # 2 q-tile pairs (halves)
LOG2E = 1.4426950408889634
# DVE Schraudolph to bf16 bits: u16 = trunc(s * A_SCH + B_SCH)
A_SCH = 128.0 * LOG2E / 8.0
B_SCH = 128.0 * 127.0 - 4.0
LAG = 2               # k-pairs between S/exp and the consuming R matmul
N_WARM = 48           # dummy matmuls to pre-warm the PE HAM clock gate
DEDUP = True          # run the ldweights dedup pass

_nc_cache = None


def _dedupe_ldweights(nc, mybir):
    """Delete InstLdweights whose (weights, position) match the live load
    on a non-clobbered array region; move their deps to the consuming
    matmul and remap references. Runs after tile scheduling, before
    nc.compile()."""
    n_del = 0
    for func in nc.m.functions:
        for block in func.blocks:
            insts = list(block.instructions)
            new = []
            live = []   # (r0, r1, c0, c1, key, name) newest last
            rename = {}
            for idx, inst in enumerate(insts):
                if isinstance(inst, mybir.InstLdweights):
                    ap = inst.ins[0]
                    tp = tuple(inst.tile_position or (0, 0))
                    ts = tuple(inst.tile_size or (128, 128))
                    key = (
                        ap.memref, ap.offset, str(ap.ap), str(ap.dtype),
                        tp, ts, str(inst.perf_mode), bool(inst.is_transpose),
                    )
                    r0, c0 = tp
                    r1, c1 = r0 + ts[0], c0 + ts[1]
                    dup = False
                    for e in reversed(live):
                        if not (r1 <= e[0] or e[1] <= r0 or c1 <= e[2] or e[3] <= c0):
                            dup = e[4] == key
                            break
                    if dup:
                        nxt = None
                        for j in range(idx + 1, len(insts)):
                            if isinstance(insts[j], mybir.InstMatmult):
                                nxt = insts[j]
                                break
                        assert nxt is not None
                        nxt.merge_dependencies_from(inst)
                        rename[inst.name] = nxt.name
                        n_del += 1
                        continue
                    # this load clobbers overlapping regions
                    live = [
                        e for e in live
                        if (r1 <= e[0] or e[1] <= r0 or c1 <= e[2] or e[3] <= c0)
                    ]
                    live.append((r0, r1, c0, c1, key, inst.name))
                new.append(inst)
            if rename:
                for inst in new:
                    inst.remap_dependency_names(rename)
                block.instructions = new
                assert len(block.instructions) == len(new)
    return n_del


def _build():
    global _nc_cache
    if _nc_cache is not None:
        return _nc_cache
    import concourse.tile as tile
    from concourse import bacc, mybir

    nc = bacc.Bacc(None, target_bir_lowering=False)
    # Keep matmul waits OFF the ldweights: the weight load then streams
    # into the array while the psum-slot wait is pending, instead of
    # serializing behind it.
    nc.move_matmul_waits_to_ldweights = lambda: None
    f32 = mybir.dt.float32
    f16 = mybir.dt.float16
    u16 = mybir.dt.uint16
    bf16 = mybir.dt.bfloat16

    kst = nc.dram_tensor("kst", [2 * D, NPAIR * KT], f16, kind="ExternalInput")
    qrep = nc.dram_tensor("qrep", [2 * D, SQ], f16, kind="ExternalInput")
    vtin = nc.dram_tensor("vtin", [KT, NKT * D], bf16, kind="ExternalInput")
    # raw R-partial halves (even k-tiles on rows 0:64, odd on 64:128)
    out_r = nc.dram_tensor("out_r", [D, SQ], f32, kind="ExternalOutput")
    # Z partial rows (from PSUM partitions 0/32/64/96); host sums + divides
    out_z = nc.dram_tensor("out_z", [2, SQ], f32, kind="ExternalOutput")

    with tile.TileContext(nc) as tc:
        with (
            tc.tile_pool(name="singles", bufs=1) as singles,
            tc.tile_pool(name="e", bufs=12) as pe_pool,
            tc.tile_pool(name="ro", bufs=2) as pro,
            tc.tile_pool(name="zo", bufs=2) as pzo,
            tc.tile_pool(name="ps_s", bufs=3, space="PSUM") as ps_s,
            tc.tile_pool(name="ps_r", bufs=1, space="PSUM") as ps_r,
            tc.tile_pool(name="ps_z", bufs=1, space="PSUM") as ps_z,
        ):
            k_sb = singles.tile([2 * D, NPAIR * KT], f16)
            q_sb = singles.tile([2 * D, SQ], f16)
            vt_sb = singles.tile([KT, NKT * D], bf16)
            bias_sb = singles.tile([KT, 1], f32)
            warm_sb = singles.tile([KT, KT], f16)
            onesb = singles.tile([KT, 1], bf16)
            # gpsimd's queue is empty at the head, so these land ~1us
            # earlier than on DVE and unblock the HAM warm-up matmuls
            nc.gpsimd.memset(warm_sb, 0.0)
            nc.gpsimd.memset(bias_sb, 0.0)
            nc.gpsimd.memset(onesb, 1.0)

            from concourse.tile_rust import add_dep_helper

            # Wave 1 (~0.6MB): first work units. Wave 2: everything else,
            # held back so its SDMA traffic doesn't delay the pipeline head.
            nc.sync.dma_start(out=q_sb[:, : 2 * QT], in_=qrep[:, : 2 * QT])
            nc.sync.dma_start(out=k_sb[:, : 4 * KT], in_=kst[:, : 4 * KT])
            d_kh = nc.gpsimd.dma_start(out=vt_sb[:, : 8 * D], in_=vtin[:, : 8 * D])
            w2 = [
                nc.sync.dma_start(out=k_sb[:, 4 * KT :], in_=kst[:, 4 * KT :]),
                nc.gpsimd.dma_start(out=vt_sb[:, 8 * D :], in_=vtin[:, 8 * D :]),
                nc.sync.dma_start(out=q_sb[:, 2 * QT :], in_=qrep[:, 2 * QT :]),
            ]
            for bulk in w2:
                add_dep_helper(
                    bulk.ins, d_kh.ins, sync=True,
                    reason="bulk input DMA after first work unit",
                )

            vt = vt_sb.rearrange("p (j d) -> p j d", j=NKT)

            # HAM warm-up: PE activity with no DMA dependency so the clock
            # gate approaches 8/8 before the real matmuls arrive. Writes into
            # the first s-psum ring slot (WAW-serialized with the real S).
            first_s = None
            if N_WARM:
                first_s = ps_s.tile([KT, 2 * QT], f32, name="s_ps")
                for _ in range(N_WARM):
                    nc.tensor.matmul(
                        first_s[:, :KT], warm_sb, warm_sb, start=True, stop=True
                    )

            Exp = mybir.ActivationFunctionType.Exp
            mult = mybir.AluOpType.mult
            add = mybir.AluOpType.add

            for h in range(NH):
                t0c = 2 * h * QT          # column base of q-tile t0
                r_ps = ps_r.tile([2 * D, QT], f32, name="r_ps")
                z_ps = ps_z.tile([KT, QT], f32, name="z_ps")
                e_of = {}

                def emit_R(gg, r_ps=r_ps, z_ps=z_ps, e_of=e_of):
                    j0 = 2 * gg
                    st = gg == 0
                    sp = gg == NPAIR - 1
                    # One R bank: t0 -> partitions 0:64 via col position 0,
                    # t1 -> 64:128 via position 64. V_even contributions
                    # start the per-element groups, V_odd accumulates on
                    # top (start=False) and carries the stop.
                    nc.tensor.matmul(
                        r_ps[0:D, :], vt[:, j0, 0:D], e_of[gg][0][:, :QT],
                        start=st, stop=False, tile_position=(0, 0),
                        skip_group_check=True,
                    )
                    nc.tensor.matmul(
                        r_ps[D : 2 * D, :], vt[:, j0, 0:D], e_of[gg][1][:, :QT],
                        start=st, stop=False, tile_position=(0, D),
                        skip_group_check=True,
                    )
                    nc.tensor.matmul(
                        r_ps[0:D, :], vt[:, j0 + 1, 0:D], e_of[gg][0][:, QT:],
                        start=False, stop=sp, tile_position=(0, 0),
                        skip_group_check=True,
                    )
                    nc.tensor.matmul(
                        r_ps[D : 2 * D, :], vt[:, j0 + 1, 0:D], e_of[gg][1][:, QT:],
                        start=False, stop=sp, tile_position=(0, D),
                        skip_group_check=True,
                    )
                    if gg % 2 == 1:
                        # Z quad over k-tiles 4b..4b+3 in one bank:
                        # positions 0/32 accumulate q-tile t0, 64/96 t1;
                        # each position chain runs 2 k-tiles sequentially
                        # off one ones-load.
                        b = gg // 2
                        zst = b == 0
                        zsp = b == NPAIR // 2 - 1
                        for i in range(4):
                            t = i // 2
                            p = 2 * b + (i % 2)
                            for half in range(2):
                                cs = slice(0, QT) if half == 0 else slice(QT, 2 * QT)
                                nc.tensor.matmul(
                                    z_ps[32 * i : 32 * i + 1, :], onesb,
                                    e_of[p][t][:, cs],
                                    start=zst and half == 0,
                                    stop=zsp and half == 1,
                                    tile_position=(0, 32 * i),
                                    skip_group_check=True,
                                )
                        del e_of[2 * b], e_of[2 * b + 1]

                for g in range(NPAIR):
                    # S pair: two (A|B) psum tiles, one per q-tile
                    if first_s is not None:
                        st0, first_s = first_s, None
                    else:
                        st0 = ps_s.tile([KT, 2 * QT], f32, name="s_ps")
                    st1 = ps_s.tile([KT, 2 * QT], f32, name="s_ps")
                    gc = slice(g * KT, (g + 1) * KT)
                    qs0 = slice(t0c, t0c + QT)
                    qs1 = slice(t0c + QT, t0c + 2 * QT)
                    nc.tensor.matmul(
                        st0[:, :QT], k_sb[0:D, gc], q_sb[0:D, qs0],
                        start=True, stop=True, tile_position=(0, 0),
                    )
                    nc.tensor.matmul(
                        st0[:, QT:], k_sb[D : 2 * D, gc], q_sb[D : 2 * D, qs0],
                        start=True, stop=True, tile_position=(D, 0),
                    )
                    nc.tensor.matmul(
                        st1[:, :QT], k_sb[0:D, gc], q_sb[0:D, qs1],
                        start=True, stop=True, tile_position=(0, 0),
                    )
                    nc.tensor.matmul(
                        st1[:, QT:], k_sb[D : 2 * D, gc], q_sb[D : 2 * D, qs1],
                        start=True, stop=True, tile_position=(D, 0),
                    )

                    # exp: one [128,1024] instruction per q-tile; fixed
                    # engines (t0 -> ACT, t1 -> DVE) so each ring slot's
                    # free-latency lines up with the PE's natural order
                    e0 = pe_pool.tile([KT, 2 * QT], bf16, tag="e")
                    e1 = pe_pool.tile([KT, 2 * QT], bf16, tag="e")
                    for t, (e_t, s_t) in enumerate(((e0, st0), (e1, st1))):
                        if t == 0:
                            nc.scalar.activation(
                                out=e_t, in_=s_t, func=Exp, scale=0.125,
                                bias=bias_sb,
                            )
                        else:
                            nc.vector.tensor_scalar(
                                e_t.bitcast(u16), s_t, A_SCH, B_SCH, mult, add
                            )
                    e_of[g] = (e0, e1)

                    if g >= LAG:
                        emit_R(g - LAG)
                for gg in range(NPAIR - LAG, NPAIR):
                    emit_R(gg)

                # evacuate R and Z (one copy each; ACT gets R, DVE Z)
                cols0 = slice(t0c, t0c + QT)
                cols1 = slice(t0c + QT, t0c + 2 * QT)
                r_sb = pro.tile([2 * D, QT], f32, tag="r_sb")
                z_sb = pzo.tile([3 * 32 + 1, QT], f32, tag="z_sb")
                nc.scalar.copy(out=r_sb, in_=r_ps)
                nc.vector.tensor_scalar(
                    z_sb, z_ps[0 : 3 * 32 + 1, :], 1.0, 0.0, mult, add
                )
                nc.sync.dma_start(out=out_r[:, cols0], in_=r_sb[0:D, :])
                nc.sync.dma_start(out=out_r[:, cols1], in_=r_sb[D : 2 * D, :])
                for i in range(4):
                    t, row = i // 2, i % 2
                    nc.sync.dma_start(
                        out=out_z[row : row + 1, cols1 if t else cols0],
                        in_=z_sb[32 * i : 32 * i + 1, :],
                    )

    if DEDUP:
        from concourse import mybir as _mb
        n = _dedupe_ldweights(nc, _mb)
        print(f"[kernel] ldweights dedup removed {n}")
    nc.compile()
    _nc_cache = nc
    return nc


def _in_maps(K, V, Q):
    K = np.asarray(K, dtype=np.float32)
    V = np.asarray(V, dtype=np.float32)
    Q = np.asarray(Q, dtype=np.float32)
    maps = []
    for c in range(NCORES):
        b, h = c // 2, c % 2
        # kst: pair-interleaved K tiles: rows 0:64 = even k-tiles, 64:128 = odd
        k16 = K[b].astype(np.float16).reshape(D, NKT, KT)
        kst = np.concatenate([k16[:, 0::2], k16[:, 1::2]], axis=0)  # [128,16,128]
        qhi = Q[b, :, h * SQ : (h + 1) * SQ].astype(np.float16)
        qrep = np.concatenate([qhi, qhi], axis=0)  # [128, SQ]
        # V'^T tiles: vt[p, j, d] = V[b, d, KT*j + p], bf16
        vtb = np.ascontiguousarray(
            V[b].T.reshape(NKT, KT, D).transpose(1, 0, 2)
        ).astype(ml_dtypes.bfloat16)
        maps.append(
            {
                "kst": np.ascontiguousarray(kst.reshape(2 * D, NPAIR * KT)),
                "qrep": np.ascontiguousarray(qrep),
                "vtin": np.ascontiguousarray(vtb.reshape(KT, NKT * D)),
            }
        )
    return maps


def _run(K, V, Q, trace=False):
    from concourse.bass_utils import run_bass_kernel_spmd

    nc = _build()
    res = run_bass_kernel_spmd(
        nc, _in_maps(K, V, Q), list(range(NCORES)), trace=trace
    )
    Q = np.asarray(Q, dtype=np.float32)
    out = np.empty((B, 2 * D, S), dtype=np.float32)
    out[:, D : 2 * D, :] = Q
    for c in range(NCORES):
        b, h = c // 2, c % 2
        rr = res.results[c]["out_r"].astype(np.float64)
        zz = res.results[c]["out_z"].astype(np.float64)
        den = zz.sum(axis=0)
        out[b, 0:D, h * SQ : (h + 1) * SQ] = (rr / den[None, :]).astype(np.float32)
    return out, res


def kernel(K, V, Q):
    out, _ = _run(K, V, Q, trace=False)
    return out


# revision 20
# speedup vs baseline: 1.2696x; 1.0459x over previous
"""Fused attention kernel for Trainium2 (Bass/Tile), SPMD over 8 NeuronCores.

Problem: B=4, D=64, S=4096 fp32 attention
    A = softmax_k(K^T Q / sqrt(D));  R = V A;  out = concat(R, Q) on channel dim.

Sharding: 8 cores = 4 batches x 2 query-halves (Sq=2048 per core).

v5 design, ~69-74us HW (v1 baseline 74.5us on a rested device):
  * k-pair-outer loop over PAIRS of q-tiles: every stationary operand
    (K-tile, V-tile, ones) serves 2+ matmuls. tile_legalize emits an
    InstLdweights per matmul unconditionally, so a post-schedule dedup
    pass deletes weight loads identical to the still-live load on a
    non-clobbered PE-array region — the extra matmuls then stream
    back-to-back on already-loaded weights (~150 loads removed).
  * S = K^T Q row-tiled 2x (k-even rows 0:64, k-odd 64:128) sharing the
    duplicated-Q stream; per-(pair,qtile) s-psum tile [128, 1024] =
    (k-even 512 | k-odd 512), ring of THREE tiles so the S -> exp -> S
    slot-recycle loop has a spare tile of slack.
  * exp: ONE [128,1024] instruction per (pair,qtile) — ACT/DVE have
    ~0.5-0.6us fixed cost per instruction, so coarse granularity wins.
    Fixed engines: t0 -> ACT (LUT exp), t1 -> DVE (one-instruction
    Schraudolph integer exp: u16 = trunc(s*16*log2e + const) = bf16
    bits, ~3% sawtooth). Scores span e^{+-16} on this dataset — beyond
    fp16's entire range — so E must be bf16 (8 exponent bits).
  * R = V E with V_even starting each per-element accumulation group
    and V_odd accumulating on top: ONE psum bank holds both q-tiles
    (t0 -> partitions 0:64 via col position 0, t1 -> 64:128 via 64),
    so no host-side half-summing and half the output DMA.
  * Z = ones^T E in ONE bank: col positions 0/32 accumulate q-tile t0,
    64/96 t1, each position chain running 2 k-tiles per ones-load.
  * No on-device softmax divide: raw R + Z partial rows out, host adds
    the 2 Z rows and divides.
  * PSUM: 3x s-tiles (2 banks each) + 1 r + 1 z = 8 banks exactly.
"""

import sys

sys.path.insert(0, "/opt/trn_rl_repo")

import numpy as np  # noqa: E402
import ml_dtypes  # noqa: E402

B, D, S = 4, 64, 4096
NCORES = 8
SQ = S * B // NCORES  # 2048 queries per core
QT = 512              # q-tile width
KT = 128              # k-tile width
NQT = SQ // QT        # 4 q-tiles per core
NKT = S // KT         # 32 k-tiles
NPAIR = NKT // 2      # 16 k-tile pairs
NH = NQT // 2         # 2 q-tile pairs (halves)
LOG2E = 1.4426950408889634
# DVE Schraudolph to bf16 bits: u16 = trunc(s * A_SCH + B_SCH)
A_SCH = 128.0 * LOG2E / 8.0
B_SCH = 128.0 * 127.0 - 4.0
LAG = 2               # k-pairs between S/exp and the consuming R matmul
N_WARM = 32           # dummy matmuls to pre-warm the PE HAM clock gate
DEDUP = True          # run the ldweights dedup pass

_nc_cache = None


def _dedupe_ldweights(nc, mybir):
    """Delete InstLdweights whose (weights, position) match the live load
    on a non-clobbered array region; move their deps to the consuming
    matmul and remap references. Runs after tile scheduling, before
    nc.compile()."""
    n_del = 0
    for func in nc.m.functions:
        for block in func.blocks:
            insts = list(block.instructions)
            new = []
            live = []   # (r0, r1, c0, c1, key, name) newest last
            rename = {}
            for idx, inst in enumerate(insts):
                if isinstance(inst, mybir.InstLdweights):
                    ap = inst.ins[0]
                    tp = tuple(inst.tile_position or (0, 0))
                    ts = tuple(inst.tile_size or (128, 128))
                    key = (
                        ap.memref, ap.offset, str(ap.ap), str(ap.dtype),
                        tp, ts, str(inst.perf_mode), bool(inst.is_transpose),
                    )
                    r0, c0 = tp
                    r1, c1 = r0 + ts[0], c0 + ts[1]
                    dup = False
                    for e in reversed(live):
                        if not (r1 <= e[0] or e[1] <= r0 or c1 <= e[2] or e[3] <= c0):
                            dup = e[4] == key
                            break
                    if dup:
                        nxt = None
                        for j in range(idx + 1, len(insts)):
                            if isinstance(insts[j], mybir.InstMatmult):
                                nxt = insts[j]
                                break
                        assert nxt is not None
                        nxt.merge_dependencies_from(inst)
                        rename[inst.name] = nxt.name
                        n_del += 1
                        continue
                    # this load clobbers overlapping regions
                    live = [
                        e for e in live
                        if (r1 <= e[0] or e[1] <= r0 or c1 <= e[2] or e[3] <= c0)
                    ]
                    live.append((r0, r1, c0, c1, key, inst.name))
                new.append(inst)
            if rename:
                for inst in new:
                    inst.remap_dependency_names(rename)
                block.instructions = new
                assert len(block.instructions) == len(new)
    return n_del


def _build():
    global _nc_cache
    if _nc_cache is not None:
        return _nc_cache
    import concourse.tile as tile
    from concourse import bacc, mybir

    nc = bacc.Bacc(None, target_bir_lowering=False)
    # Keep matmul waits OFF the ldweights: the weight load then streams
    # into the array while the psum-slot wait is pending, instead of
    # serializing behind it.
    nc.move_matmul_waits_to_ldweights = lambda: None
    f32 = mybir.dt.float32
    f16 = mybir.dt.float16
    u16 = mybir.dt.uint16
    bf16 = mybir.dt.bfloat16

    kst = nc.dram_tensor("kst", [2 * D, NPAIR * KT], f16, kind="ExternalInput")
    qrep = nc.dram_tensor("qrep", [2 * D, SQ], f16, kind="ExternalInput")
    vtin = nc.dram_tensor("vtin", [KT, NKT * D], bf16, kind="ExternalInput")
    # raw R-partial halves (even k-tiles on rows 0:64, odd on 64:128)
    out_r = nc.dram_tensor("out_r", [D, SQ], f32, kind="ExternalOutput")
    # Z partial rows (from PSUM partitions 0/32/64/96); host sums + divides
    out_z = nc.dram_tensor("out_z", [2, SQ], f32, kind="ExternalOutput")

    with tile.TileContext(nc) as tc:
        with (
            tc.tile_pool(name="singles", bufs=1) as singles,
            tc.tile_pool(name="e", bufs=12) as pe_pool,
            tc.tile_pool(name="ro", bufs=2) as pro,
            tc.tile_pool(name="zo", bufs=2) as pzo,
            tc.tile_pool(name="ps_s", bufs=3, space="PSUM") as ps_s,
            tc.tile_pool(name="ps_r", bufs=1, space="PSUM") as ps_r,
            tc.tile_pool(name="ps_z", bufs=1, space="PSUM") as ps_z,
        ):
            k_sb = singles.tile([2 * D, NPAIR * KT], f16)
            q_sb = singles.tile([2 * D, SQ], f16)
            vt_sb = singles.tile([KT, NKT * D], bf16)
            bias_sb = singles.tile([KT, 1], f32)
            warm_sb = singles.tile([KT, KT], f16)
            onesb = singles.tile([KT, 1], bf16)
            nc.vector.memset(bias_sb, 0.0)
            nc.vector.memset(warm_sb, 0.0)
            nc.vector.memset(onesb, 1.0)

            from concourse.tile_rust import add_dep_helper

            # Wave 1 (~0.6MB): first work units. Wave 2: everything else,
            # held back so its SDMA traffic doesn't delay the pipeline head.
            nc.sync.dma_start(out=q_sb[:, : 2 * QT], in_=qrep[:, : 2 * QT])
            nc.sync.dma_start(out=k_sb[:, : 4 * KT], in_=kst[:, : 4 * KT])
            d_kh = nc.gpsimd.dma_start(out=vt_sb[:, : 8 * D], in_=vtin[:, : 8 * D])
            w2 = [
                nc.sync.dma_start(out=k_sb[:, 4 * KT :], in_=kst[:, 4 * KT :]),
                nc.gpsimd.dma_start(out=vt_sb[:, 8 * D :], in_=vtin[:, 8 * D :]),
                nc.sync.dma_start(out=q_sb[:, 2 * QT :], in_=qrep[:, 2 * QT :]),
            ]
            for bulk in w2:
                add_dep_helper(
                    bulk.ins, d_kh.ins, sync=True,
                    reason="bulk input DMA after first work unit",
                )

            vt = vt_sb.rearrange("p (j d) -> p j d", j=NKT)

            # HAM warm-up: PE activity with no DMA dependency so the clock
            # gate approaches 8/8 before the real matmuls arrive. Writes into
            # the first s-psum ring slot (WAW-serialized with the real S).
            first_s = None
            if N_WARM:
                first_s = ps_s.tile([KT, 2 * QT], f32, name="s_ps")
                for _ in range(N_WARM):
                    nc.tensor.matmul(
                        first_s[:, :KT], warm_sb, warm_sb, start=True, stop=True
                    )

            Exp = mybir.ActivationFunctionType.Exp
            mult = mybir.AluOpType.mult
            add = mybir.AluOpType.add

            for h in range(NH):
                t0c = 2 * h * QT          # column base of q-tile t0
                r_ps = ps_r.tile([2 * D, QT], f32, name="r_ps")
                z_ps = ps_z.tile([KT, QT], f32, name="z_ps")
                e_of = {}

                def emit_R(gg, r_ps=r_ps, z_ps=z_ps, e_of=e_of):
                    j0 = 2 * gg
                    st = gg == 0
                    sp = gg == NPAIR - 1
                    # One R bank: t0 -> partitions 0:64 via col position 0,
                    # t1 -> 64:128 via position 64. V_even contributions
                    # start the per-element groups, V_odd accumulates on
                    # top (start=False) and carries the stop.
                    nc.tensor.matmul(
                        r_ps[0:D, :], vt[:, j0, 0:D], e_of[gg][0][:, :QT],
                        start=st, stop=False, tile_position=(0, 0),
                        skip_group_check=True,
                    )
                    nc.tensor.matmul(
                        r_ps[D : 2 * D, :], vt[:, j0, 0:D], e_of[gg][1][:, :QT],
                        start=st, stop=False, tile_position=(0, D),
                        skip_group_check=True,
                    )
                    nc.tensor.matmul(
                        r_ps[0:D, :], vt[:, j0 + 1, 0:D], e_of[gg][0][:, QT:],
                        start=False, stop=sp, tile_position=(0, 0),
                        skip_group_check=True,
                    )
                    nc.tensor.matmul(
                        r_ps[D : 2 * D, :], vt[:, j0 + 1, 0:D], e_of[gg][1][:, QT:],
                        start=False, stop=sp, tile_position=(0, D),
                        skip_group_check=True,
                    )
                    if gg % 2 == 1:
                        # Z quad over k-tiles 4b..4b+3 in one bank:
                        # positions 0/32 accumulate q-tile t0, 64/96 t1;
                        # each position chain runs 2 k-tiles sequentially
                        # off one ones-load.
                        b = gg // 2
                        zst = b == 0
                        zsp = b == NPAIR // 2 - 1
                        for i in range(4):
                            t = i // 2
                            p = 2 * b + (i % 2)
                            for half in range(2):
                                cs = slice(0, QT) if half == 0 else slice(QT, 2 * QT)
                                nc.tensor.matmul(
                                    z_ps[32 * i : 32 * i + 1, :], onesb,
                                    e_of[p][t][:, cs],
                                    start=zst and half == 0,
                                    stop=zsp and half == 1,
                                    tile_position=(0, 32 * i),
                                    skip_group_check=True,
                                )
                        del e_of[2 * b], e_of[2 * b + 1]

                for g in range(NPAIR):
                    # S pair: two (A|B) psum tiles, one per q-tile
                    if first_s is not None:
                        st0, first_s = first_s, None
                    else:
                        st0 = ps_s.tile([KT, 2 * QT], f32, name="s_ps")
                    st1 = ps_s.tile([KT, 2 * QT], f32, name="s_ps")
                    gc = slice(g * KT, (g + 1) * KT)
                    qs0 = slice(t0c, t0c + QT)
                    qs1 = slice(t0c + QT, t0c + 2 * QT)
                    nc.tensor.matmul(
                        st0[:, :QT], k_sb[0:D, gc], q_sb[0:D, qs0],
                        start=True, stop=True, tile_position=(0, 0),
                    )
                    nc.tensor.matmul(
                        st0[:, QT:], k_sb[D : 2 * D, gc], q_sb[D : 2 * D, qs0],
                        start=True, stop=True, tile_position=(D, 0),
                    )
                    nc.tensor.matmul(
                        st1[:, :QT], k_sb[0:D, gc], q_sb[0:D, qs1],
                        start=True, stop=True, tile_position=(0, 0),
                    )
                    nc.tensor.matmul(
                        st1[:, QT:], k_sb[D : 2 * D, gc], q_sb[D : 2 * D, qs1],
                        start=True, stop=True, tile_position=(D, 0),
                    )

                    # exp: one [128,1024] instruction per q-tile; fixed
                    # engines (t0 -> ACT, t1 -> DVE) so each ring slot's
                    # free-latency lines up with the PE's natural order
                    e0 = pe_pool.tile([KT, 2 * QT], bf16, tag="e")
                    e1 = pe_pool.tile([KT, 2 * QT], bf16, tag="e")
                    for t, (e_t, s_t) in enumerate(((e0, st0), (e1, st1))):
                        if t == 0:
                            nc.scalar.activation(
                                out=e_t, in_=s_t, func=Exp, scale=0.125,
                                bias=bias_sb,
                            )
                        else:
                            nc.vector.tensor_scalar(
                                e_t.bitcast(u16), s_t, A_SCH, B_SCH, mult, add
                            )
                    e_of[g] = (e0, e1)

                    if g >= LAG:
                        emit_R(g - LAG)
                for gg in range(NPAIR - LAG, NPAIR):
                    emit_R(gg)

                # evacuate R and Z (one copy each; ACT gets R, DVE Z)
                cols0 = slice(t0c, t0c + QT)
                cols1 = slice(t0c + QT, t0c + 2 * QT)
                r_sb = pro.tile([2 * D, QT], f32, tag="r_sb")
                z_sb = pzo.tile([3 * 32 + 1, QT], f32, tag="z_sb")
                nc.scalar.copy(out=r_sb, in_=r_ps)
                nc.vector.tensor_scalar(
                    z_sb, z_ps[0 : 3 * 32 + 1, :], 1.0, 0.0, mult, add
                )
                nc.sync.dma_start(out=out_r[:, cols0], in_=r_sb[0:D, :])
                nc.sync.dma_start(out=out_r[:, cols1], in_=r_sb[D : 2 * D, :])
                for i in range(4):
                    t, row = i // 2, i % 2
                    nc.sync.dma_start(
                        out=out_z[row : row + 1, cols1 if t else cols0],
                        in_=z_sb[32 * i : 32 * i + 1, :],
                    )

    if DEDUP:
        from concourse import mybir as _mb
        n = _dedupe_ldweights(nc, _mb)
        print(f"[kernel] ldweights dedup removed {n}")
    nc.compile()
    _nc_cache = nc
    return nc


def _in_maps(K, V, Q):
    K = np.asarray(K, dtype=np.float32)
    V = np.asarray(V, dtype=np.float32)
    Q = np.asarray(Q, dtype=np.float32)
    maps = []
    for c in range(NCORES):
        b, h = c // 2, c % 2
        # kst: pair-interleaved K tiles: rows 0:64 = even k-tiles, 64:128 = odd
        k16 = K[b].astype(np.float16).reshape(D, NKT, KT)
        kst = np.concatenate([k16[:, 0::2], k16[:, 1::2]], axis=0)  # [128,16,128]
        qhi = Q[b, :, h * SQ : (h + 1) * SQ].astype(np.float16)
        qrep = np.concatenate([qhi, qhi], axis=0)  # [128, SQ]
        # V'^T tiles: vt[p, j, d] = V[b, d, KT*j + p], bf16
        vtb = np.ascontiguousarray(
            V[b].T.reshape(NKT, KT, D).transpose(1, 0, 2)
        ).astype(ml_dtypes.bfloat16)
        maps.append(
            {
                "kst": np.ascontiguousarray(kst.reshape(2 * D, NPAIR * KT)),
                "qrep": np.ascontiguousarray(qrep),
                "vtin": np.ascontiguousarray(vtb.reshape(KT, NKT * D)),
            }
        )
    return maps


def _run(K, V, Q, trace=False):
    from concourse.bass_utils import run_bass_kernel_spmd

    nc = _build()
    res = run_bass_kernel_spmd(
        nc, _in_maps(K, V, Q), list(range(NCORES)), trace=trace
    )
    Q = np.asarray(Q, dtype=np.float32)
    out = np.empty((B, 2 * D, S), dtype=np.float32)
    out[:, D : 2 * D, :] = Q
    for c in range(NCORES):
        b, h = c // 2, c % 2
        rr = res.results[c]["out_r"].astype(np.float64)
        zz = res.results[c]["out_z"].astype(np.float64)
        den = zz.sum(axis=0)
        out[b, 0:D, h * SQ : (h + 1) * SQ] = (rr / den[None, :]).astype(np.float32)
    return out, res


def kernel(K, V, Q):
    out, _ = _run(K, V, Q, trace=False)
    return out
